# revision 5
# baseline (speedup 1.0000x reference)
"""Trainium2 Bass kernel for SAGAN-style spatial self-attention.

Reference computation (per batch b):
    xf = x[b].reshape(C, N)                    # C=256, N=64*64=4096
    f  = w1 @ xf                               # [32, N]   (query^T)
    g  = w2 @ xf                               # [32, N]   (key)
    V  = (w3 @ xf)^T                           # [N, C]    (value)
    S  = f^T @ g                               # [N, N]
    O  = softmax(S, axis=-1) @ V               # [N, C]
    out[b] = O^T.reshape(C, H, W) + x[b]

Sharding: 8 cores = 4 batches x 2 query-halves. Each core holds its batch's
full xf (for keys/values) and computes attention for 2048 query positions.
No cross-core communication.

Host-side, each core's key axis is permuted so its own 2048 query columns
come first: the f-projection then reads the same SBUF tile as g/V (no
separate xq upload), and softmax/PV are key-permutation invariant.

Per-core device algorithm (n = this core's 2048 query cols, m = all 4096 keys):
  - projections f [32,2048], g [32,4096]; V [4096,257]
    (column 256 of V is ones -> PV matmul emits softmax denominator for free)
  - S^T chunks: matmul(lhsT=g_mtile [32,128], rhs=f_chunk [32,512]) -> PSUM
  - P^T = exp(S^T) -> SBUF bf16 (no max subtraction: |S| <~ 45, exp fits
    fp32 and bf16 shares the fp32 exponent range)
  - O chunk: matmul(lhsT=P^T [128m,128n], rhs=V [128m,257]) accumulated over
    32 m-tiles -> [128n, 257]; r = 1/col256; O *= r
  - residual add in [n, C] layout, fp16 output in a [128, J, C] DRAM layout
    (16KB contiguous per partition row -> full-rate output DMA packets);
    the [C, n] transpose happens in the host-side gather.

All PE operands are bf16: the PE drains its pipeline on an operand-dtype
switch (~100-200ns each), and the PV<->S^T interleave would otherwise pay
that at every border. bf16 scores cost ~4e-3 rel err (gate 2e-2). fp32
matmuls are additionally avoided because they do not register as PE-busy
for the HAM clock gate (PE stuck at 1.2GHz).

DMA: few, large descriptors (descriptor issue costs ~0.6us each on the
issuing queue), split across the sync and scalar queues so the projection
pipeline starts as early as possible. S^T groups are emitted in adjacent
pairs so the PE pays the PV<->S^T transition penalty half as often.
"""

import sys

sys.path.insert(0, "/opt/trn_rl_repo")

from contextlib import ExitStack

import ml_dtypes
import numpy as np

import concourse.bass as bass
import concourse.tile as tile
from concourse import bacc, mybir
from concourse.bass import ts, ds
from concourse.bass_utils import run_bass_kernel_spmd

F32 = mybir.dt.float32
F16 = mybir.dt.float16
BF16 = mybir.dt.bfloat16
BF = ml_dtypes.bfloat16

B, C, H, W = 4, 256, 64, 64
N = H * W          # 4096 keys per batch
NQ = N // 2        # 2048 queries per core
CK = 32            # query/key head dim
MT = N // 128      # 32 m-tiles
NCHUNK = NQ // 512  # 4 n-chunks of 512 query cols
NJ = NQ // 128     # 16 output row-tiles
N_WARM = 5         # HAM warmup matmuls (PE clock ramp) while input DMAs land
XSPLIT = 1536      # xkv piece split: [0:1536] covers the f-projection range
EXP = mybir.ActivationFunctionType.Exp


def build_nc():
    nc = bacc.Bacc("TRN2", target_bir_lowering=False, debug=False, num_devices=8)
    xkv_d = nc.dram_tensor("xkv", [C, N], BF16, kind="ExternalInput")
    xqt_d = nc.dram_tensor("xqt", [128, NJ, C], F16, kind="ExternalInput")
    w12_d = nc.dram_tensor("w12", [C, 2 * CK], BF16, kind="ExternalInput")
    w3t_d = nc.dram_tensor("w3t", [C, C], BF16, kind="ExternalInput")
    out_d = nc.dram_tensor("out", [128, NJ, C], F16, kind="ExternalOutput")

    with tile.TileContext(nc) as tc, ExitStack() as ctx:
        _body(ctx, tc, xkv_d.ap(), xqt_d.ap(), w12_d.ap(), w3t_d.ap(),
              out_d.ap())
    nc.compile()
    return nc


def _body(ctx, tc, xkv_d, xqt_d, w12_d, w3t_d, out_d):
    nc = tc.nc
    singles = ctx.enter_context(tc.tile_pool(name="singles", bufs=1))

    xqt = singles.tile([128, NJ, C], F16, tag="xqt", name="xqt")
    xkv_h = singles.tile([128, 2, N], BF16, tag="xkv_h", name="xkv_h")
    w12t = singles.tile([128, 2, 2 * CK], BF16, tag="w12t", name="w12t")
    w3t = singles.tile([128, 2, C], BF16, tag="w3t", name="w3t")
    g_sb = singles.tile([CK, N], BF16, tag="g_sb", name="g_sb")
    f_sb = singles.tile([CK, NQ], BF16, tag="f_sb", name="f_sb")
    V = singles.tile([128, MT, 260], BF16, tag="V", name="V")
    warm = singles.tile([128, 512], BF16, tag="warm", name="warm")

    nc.vector.memset(warm[:], 0.0)
    nc.vector.memset(V[:, :, 256:257], 1.0)

    # PSUM: the S^T pool (2-bank slots, bufs=2) + a 1-bank pool (bufs=4) for
    # the PV accumulators and all projection outputs. 4 + 4 = 8 banks.
    stp = ctx.enter_context(tc.tile_pool(name="st_ps", bufs=2, space="PSUM"))
    op = ctx.enter_context(tc.tile_pool(name="o_ps", bufs=4, space="PSUM"))
    ptp = ctx.enter_context(tc.tile_pool(name="pt", bufs=2))
    rp = ctx.enter_context(tc.tile_pool(name="r", bufs=2))
    stgp = ctx.enter_context(tc.tile_pool(name="stage", bufs=3))

    Pt = [None, None]
    stage = [None, None]
    posts = []

    def emit_post(item):
        cc, j, o_ps, stg = item
        J = cc * 4 + j
        r = rp.tile([128, 1], F32, tag="r", name="r")
        if cc < NCHUNK - 1:
            nc.vector.reciprocal(r[:], o_ps[:, 256:257])
            nc.vector.tensor_scalar_mul(stg[:, j, :], o_ps[:, 0:256], r[:])
            nc.vector.tensor_add(stg[:, j, :], stg[:, j, :], xqt[:, J, :])
            if j == 3:
                nc.sync.dma_start(out_d[:, 4 * cc:4 * cc + 4, :], stg[:, :, :])
        elif j < 3:
            # final chunk: ACT is idle (no exps left) — do the normalize
            # there and ship each row-tile as soon as it is ready
            nc.vector.reciprocal(r[:], o_ps[:, 256:257])
            nc.scalar.mul(stg[:, j, :], o_ps[:, 0:256], r[:])
            nc.vector.tensor_add(stg[:, j, :], stg[:, j, :], xqt[:, J, :])
            eng = (nc.sync, nc.scalar, nc.sync)[j]
            eng.dma_start(out_d[:, 4 * cc + j, :], stg[:, j, :])
        else:
            # last row-tile: partition-split pipeline across DVE/ACT and
            # two DMA queues so the post-matmul drain stays under ~2us
            nc.vector.reciprocal(r[:], o_ps[:, 256:257])
            Jf = 4 * cc + 3
            for h, eng in ((0, nc.scalar), (1, nc.sync)):
                hp = ds(64 * h, 64)
                nc.scalar.mul(stg[hp, j, :], o_ps[hp, 0:256], r[hp, :])
                nc.vector.tensor_add(stg[hp, j, :], stg[hp, j, :],
                                     xqt[hp, J, :])
                eng.dma_start(out_d[hp, Jf, :], stg[hp, 3, :])

    def st_mm(st, c, gidx, t):
        mt = 2 * gidx + t
        nc.tensor.matmul(st[:, t, :], g_sb[:, ts(mt, 128)],
                         f_sb[:, ts(c, 512)], start=True, stop=True)

    def st_group(c, gidx):
        st = stp.tile([128, 2, 512], F32, tag="st", name="st")
        st_mm(st, c, gidx, 0)
        st_mm(st, c, gidx, 1)
        nc.scalar.activation(Pt[c % 2][:, 2 * gidx:2 * gidx + 2, :], st[:], EXP)

    def f_proj(ch):
        fp = op.tile([CK, 512], F32, tag="o", name="fp")
        for k in range(2):
            nc.tensor.matmul(fp[:], w12t[:, k, 0:CK], xkv_h[:, k, ts(ch, 512)],
                             start=(k == 0), stop=(k == 1))
        nc.vector.tensor_copy(f_sb[:, ts(ch, 512)], fp[:])

    # HAM warmup: the PE clock-gate opens only after ~3.4us of gapless
    # streaming; run a short dummy bf16 burst while the first input DMAs
    # land, then roll straight into the (real) projection stream.
    wps = stp.tile([128, 2, 512], F32, tag="st", name="wps")
    for i in range(N_WARM):
        nc.tensor.matmul(wps[:, i % 2, :], warm[:, 0:128], warm[:],
                         start=True, stop=True)

    # ---- input DMAs (bf16 operands are cast host-side) ----
    # sync queue: xkv k-half 0; scalar queue: weights, xkv k-half 1, then
    # the residual (not needed until the first posts ~30us in)
    nc.sync.dma_start(xkv_h[:, 0, 0:XSPLIT], xkv_d[ts(0, 128), 0:XSPLIT])
    nc.sync.dma_start(xkv_h[:, 0, XSPLIT:N], xkv_d[ts(0, 128), XSPLIT:N])
    nc.scalar.dma_start(w12t[:, 0, :], w12_d[ts(0, 128), :])
    nc.scalar.dma_start(w12t[:, 1, :], w12_d[ts(1, 128), :])
    nc.scalar.dma_start(xkv_h[:, 1, 0:XSPLIT], xkv_d[ts(1, 128), 0:XSPLIT])
    nc.scalar.dma_start(w3t[:, 0, :], w3t_d[ts(0, 128), :])
    nc.scalar.dma_start(w3t[:, 1, :], w3t_d[ts(1, 128), :])
    nc.scalar.dma_start(xkv_h[:, 1, XSPLIT:N], xkv_d[ts(1, 128), XSPLIT:N])
    nc.scalar.dma_start(xqt[:], xqt_d[:])

    # ---- projections: f leads (queries = first 4 key chunks), then per
    # 512-col chunk one cycle of [S^T pair (lagging), g, V x4]; the f tail
    # is folded into the first loop iterations. S^T lags g by one chunk so
    # the PE never waits on the DVE g-copy; the V tiles keep the PE dense
    # while ACT drains the exps.
    Pt[0] = ptp.tile([128, MT, 512], BF16, tag="pt", name="pt")
    f_proj(0)
    f_proj(1)
    for ch in range(N // 512):
        if ch >= 1:
            st_group(0, 2 * (ch - 1))
            st_group(0, 2 * ch - 1)
        gp = op.tile([CK, 512], F32, tag="o", name="gp")
        for k in range(2):
            nc.tensor.matmul(gp[:], w12t[:, k, CK:2 * CK],
                             xkv_h[:, k, ts(ch, 512)],
                             start=(k == 0), stop=(k == 1))
        nc.vector.tensor_copy(g_sb[:, ts(ch, 512)], gp[:])
        if ch < 2:
            f_proj(ch + 2)
        for mt in range(4 * ch, 4 * ch + 4):
            vp = op.tile([128, 256], F32, tag="o", name="vp")
            for k in range(2):
                nc.tensor.matmul(vp[:], xkv_h[:, k, ts(mt, 128)], w3t[:, k, :],
                                 start=(k == 0), stop=(k == 1))
            nc.vector.tensor_copy(V[:, mt, 0:256], vp[:])
    st_group(0, 14)
    st_group(0, 15)

    # ---- attention chunks 1..NCHUNK, software-pipelined by one chunk ----
    for c in range(1, NCHUNK + 1):
        if c < NCHUNK:
            Pt[c % 2] = ptp.tile([128, MT, 512], BF16, tag="pt", name="pt")
        stage[(c - 1) % 2] = stgp.tile([128, 4, 256], F16, tag="stage", name="stage")
        o_cur = None
        for gidx in range(16):
            j, seg = gidx // 4, gidx % 4
            if seg == 0:
                o_cur = op.tile([128, 257], F32, tag="o", name="o")
            for mm in range(4):
                mt = seg * 8 + mm
                nc.tensor.matmul(o_cur[:], Pt[(c - 1) % 2][:, mt, ts(j, 128)],
                                 V[:, mt, 0:257],
                                 start=(mt == 0), stop=(mt == MT - 1),
                                 skip_group_check=True)
            # S^T groups in adjacent pairs: half as many PV<->S^T stream
            # transitions on the PE (each costs ~100ns of drained pipeline)
            if c < NCHUNK and gidx % 2 == 0:
                st_group(c, gidx)
                st_group(c, gidx + 1)
            for mm in range(4, 8):
                mt = seg * 8 + mm
                nc.tensor.matmul(o_cur[:], Pt[(c - 1) % 2][:, mt, ts(j, 128)],
                                 V[:, mt, 0:257],
                                 start=(mt == 0), stop=(mt == MT - 1),
                                 skip_group_check=True)
            if seg == 3:
                posts.append((c - 1, j, o_cur, stage[(c - 1) % 2]))
            # delay each n-tile's post-processing by one PE group so the DVE
            # normalize never stalls the PE stream; the final chunk has no
            # S^T stream left to protect, so flush immediately there
            while len(posts) > (1 if (gidx < 15 and c < NCHUNK) else 0):
                emit_post(posts.pop(0))
    while posts:
        emit_post(posts.pop(0))


_NC_CACHE = None


def _get_nc():
    global _NC_CACHE
    if _NC_CACHE is None:
        _NC_CACHE = build_nc()
    return _NC_CACHE


def make_in_maps(x, w1, w2, w3):
    x = np.ascontiguousarray(x, dtype=np.float32).reshape(B, C, N)
    w12 = np.ascontiguousarray(np.concatenate([w1.T, w2.T], axis=1)).astype(BF)
    w3t = np.ascontiguousarray(w3.T).astype(BF)
    in_maps = []
    xh = x.astype(BF)
    for core in range(8):
        b, half = core // 2, core % 2
        qsl = slice(half * NQ, (half + 1) * NQ)
        osl = slice((1 - half) * NQ, (2 - half) * NQ)
        # key axis permuted: own query columns first
        xkv = np.concatenate([xh[b][:, qsl], xh[b][:, osl]], axis=1)
        # residual for this core's queries: [NQ, C] -> [128, NJ, C]
        xqt = np.ascontiguousarray(
            x[b][:, qsl].T.astype(np.float16)
            .reshape(NJ, 128, C).transpose(1, 0, 2))
        in_maps.append({
            "xkv": np.ascontiguousarray(xkv),
            "xqt": xqt,
            "w12": w12,
            "w3t": w3t,
        })
    return in_maps


def assemble(results):
    out = np.empty((B, C, N), dtype=np.float32)
    for core in range(8):
        b, half = core // 2, core % 2
        o = np.asarray(results[core]["out"], dtype=np.float32)  # [128, NJ, C]
        out[b][:, half * NQ:(half + 1) * NQ] = (
            o.transpose(1, 0, 2).reshape(NQ, C).T)
    return out.reshape(B, C, H, W)


def kernel(x, w1, w2, w3):
    nc = _get_nc()
    res = run_bass_kernel_spmd(nc, make_in_maps(x, w1, w2, w3),
                               core_ids=list(range(8)))
    return assemble(res.results)


# revision 6
# speedup vs baseline: 1.0012x; 1.0012x over previous
"""Trainium2 Bass kernel for SAGAN-style spatial self-attention.

Reference computation (per batch b):
    xf = x[b].reshape(C, N)                    # C=256, N=64*64=4096
    f  = w1 @ xf                               # [32, N]   (query^T)
    g  = w2 @ xf                               # [32, N]   (key)
    V  = (w3 @ xf)^T                           # [N, C]    (value)
    S  = f^T @ g                               # [N, N]
    O  = softmax(S, axis=-1) @ V               # [N, C]
    out[b] = O^T.reshape(C, H, W) + x[b]

Sharding: 8 cores = 4 batches x 2 query-halves. Each core holds its batch's
full xf (for keys/values) and computes attention for 2048 query positions.
No cross-core communication.

Host-side, each core's key axis is permuted so its own 2048 query columns
come first: the f-projection then reads the same SBUF tile as g/V (no
separate xq upload), and softmax/PV are key-permutation invariant.

Per-core device algorithm (n = this core's 2048 query cols, m = all 4096 keys):
  - projections f [32,2048], g [32,4096]; V [4096,257]
    (column 256 of V is ones -> PV matmul emits softmax denominator for free)
  - S^T chunks: matmul(lhsT=g_mtile [32,128], rhs=f_chunk [32,512]) -> PSUM
  - P^T = exp(S^T) -> SBUF bf16 (no max subtraction: |S| <~ 45, exp fits
    fp32 and bf16 shares the fp32 exponent range)
  - O chunk: matmul(lhsT=P^T [128m,128n], rhs=V [128m,257]) accumulated over
    32 m-tiles -> [128n, 257]; r = 1/col256; O *= r
  - residual add in [n, C] layout, fp16 output in a [128, J, C] DRAM layout
    (16KB contiguous per partition row -> full-rate output DMA packets);
    the [C, n] transpose happens in the host-side gather.

All PE operands are bf16: the PE drains its pipeline on an operand-dtype
switch (~100-200ns each), and the PV<->S^T interleave would otherwise pay
that at every border. bf16 scores cost ~4e-3 rel err (gate 2e-2). fp32
matmuls are additionally avoided because they do not register as PE-busy
for the HAM clock gate (PE stuck at 1.2GHz).

DMA: few, large descriptors (descriptor issue costs ~0.6us each on the
issuing queue), split across the sync and scalar queues so the projection
pipeline starts as early as possible. S^T groups are emitted in adjacent
pairs so the PE pays the PV<->S^T transition penalty half as often.
"""

import sys

sys.path.insert(0, "/opt/trn_rl_repo")

from contextlib import ExitStack

import ml_dtypes
import numpy as np

import concourse.bass as bass
import concourse.tile as tile
from concourse import bacc, mybir
from concourse.bass import ts, ds
from concourse.bass_utils import run_bass_kernel_spmd

F32 = mybir.dt.float32
F16 = mybir.dt.float16
BF16 = mybir.dt.bfloat16
BF = ml_dtypes.bfloat16

B, C, H, W = 4, 256, 64, 64
N = H * W          # 4096 keys per batch
NQ = N // 2        # 2048 queries per core
CK = 32            # query/key head dim
MT = N // 128      # 32 m-tiles
NCHUNK = NQ // 512  # 4 n-chunks of 512 query cols
NJ = NQ // 128     # 16 output row-tiles
N_WARM = 7         # HAM warmup matmuls (PE clock ramp) while input DMAs land
XSPLIT = 1536      # xkv piece split: [0:1536] covers the f-projection range
EXP = mybir.ActivationFunctionType.Exp


def build_nc():
    nc = bacc.Bacc("TRN2", target_bir_lowering=False, debug=False, num_devices=8)
    xkv_d = nc.dram_tensor("xkv", [C, N], BF16, kind="ExternalInput")
    xqt_d = nc.dram_tensor("xqt", [128, NJ, C], F16, kind="ExternalInput")
    w12_d = nc.dram_tensor("w12", [C, 2 * CK], BF16, kind="ExternalInput")
    w3t_d = nc.dram_tensor("w3t", [C, C], BF16, kind="ExternalInput")
    out_d = nc.dram_tensor("out", [128, NJ, C], F16, kind="ExternalOutput")

    with tile.TileContext(nc) as tc, ExitStack() as ctx:
        _body(ctx, tc, xkv_d.ap(), xqt_d.ap(), w12_d.ap(), w3t_d.ap(),
              out_d.ap())
    nc.compile()
    return nc


def _body(ctx, tc, xkv_d, xqt_d, w12_d, w3t_d, out_d):
    nc = tc.nc
    singles = ctx.enter_context(tc.tile_pool(name="singles", bufs=1))

    xqt = singles.tile([128, NJ, C], F16, tag="xqt", name="xqt")
    xkv_h = singles.tile([128, 2, N], BF16, tag="xkv_h", name="xkv_h")
    w12t = singles.tile([128, 2, 2 * CK], BF16, tag="w12t", name="w12t")
    w3t = singles.tile([128, 2, C], BF16, tag="w3t", name="w3t")
    g_sb = singles.tile([CK, N], BF16, tag="g_sb", name="g_sb")
    f_sb = singles.tile([CK, NQ], BF16, tag="f_sb", name="f_sb")
    V = singles.tile([128, MT, 260], BF16, tag="V", name="V")
    warm = singles.tile([128, 512], BF16, tag="warm", name="warm")

    nc.vector.memset(warm[:], 0.0)
    nc.vector.memset(V[:, :, 256:257], 1.0)

    # PSUM: the S^T pool (2-bank slots, bufs=2) + a 1-bank pool (bufs=4) for
    # the PV accumulators and all projection outputs. 4 + 4 = 8 banks.
    stp = ctx.enter_context(tc.tile_pool(name="st_ps", bufs=2, space="PSUM"))
    op = ctx.enter_context(tc.tile_pool(name="o_ps", bufs=4, space="PSUM"))
    ptp = ctx.enter_context(tc.tile_pool(name="pt", bufs=2))
    rp = ctx.enter_context(tc.tile_pool(name="r", bufs=2))
    stgp = ctx.enter_context(tc.tile_pool(name="stage", bufs=3))

    Pt = [None, None]
    stage = [None, None]
    posts = []

    def emit_post(item):
        cc, j, o_ps, stg = item
        J = cc * 4 + j
        r = rp.tile([128, 1], F32, tag="r", name="r")
        if cc < NCHUNK - 1:
            nc.vector.reciprocal(r[:], o_ps[:, 256:257])
            nc.vector.tensor_scalar_mul(stg[:, j, :], o_ps[:, 0:256], r[:])
            nc.vector.tensor_add(stg[:, j, :], stg[:, j, :], xqt[:, J, :])
            if j == 3:
                nc.sync.dma_start(out_d[:, 4 * cc:4 * cc + 4, :], stg[:, :, :])
        elif j < 3:
            # final chunk: ACT is idle (no exps left) — do the normalize
            # there and ship each row-tile as soon as it is ready
            nc.vector.reciprocal(r[:], o_ps[:, 256:257])
            nc.scalar.mul(stg[:, j, :], o_ps[:, 0:256], r[:])
            nc.vector.tensor_add(stg[:, j, :], stg[:, j, :], xqt[:, J, :])
            eng = (nc.sync, nc.scalar, nc.sync)[j]
            eng.dma_start(out_d[:, 4 * cc + j, :], stg[:, j, :])
        else:
            # last row-tile: partition-split pipeline across DVE/ACT and
            # two DMA queues so the post-matmul drain stays under ~2us
            nc.vector.reciprocal(r[:], o_ps[:, 256:257])
            Jf = 4 * cc + 3
            for h, eng in ((0, nc.scalar), (1, nc.sync)):
                hp = ds(64 * h, 64)
                nc.scalar.mul(stg[hp, j, :], o_ps[hp, 0:256], r[hp, :])
                nc.vector.tensor_add(stg[hp, j, :], stg[hp, j, :],
                                     xqt[hp, J, :])
                eng.dma_start(out_d[hp, Jf, :], stg[hp, 3, :])

    def st_mm(st, c, gidx, t):
        mt = 2 * gidx + t
        nc.tensor.matmul(st[:, t, :], g_sb[:, ts(mt, 128)],
                         f_sb[:, ts(c, 512)], start=True, stop=True)

    def st_group(c, gidx):
        st = stp.tile([128, 2, 512], F32, tag="st", name="st")
        st_mm(st, c, gidx, 0)
        st_mm(st, c, gidx, 1)
        nc.scalar.activation(Pt[c % 2][:, 2 * gidx:2 * gidx + 2, :], st[:], EXP)

    def f_proj(ch):
        fp = op.tile([CK, 512], F32, tag="o", name="fp")
        for k in range(2):
            nc.tensor.matmul(fp[:], w12t[:, k, 0:CK], xkv_h[:, k, ts(ch, 512)],
                             start=(k == 0), stop=(k == 1))
        nc.vector.tensor_copy(f_sb[:, ts(ch, 512)], fp[:])

    # HAM warmup: the PE clock-gate opens only after ~3.4us of gapless
    # streaming; run a short dummy bf16 burst while the first input DMAs
    # land, then roll straight into the (real) projection stream.
    wps = stp.tile([128, 2, 512], F32, tag="st", name="wps")
    for i in range(N_WARM):
        nc.tensor.matmul(wps[:, i % 2, :], warm[:, 0:128], warm[:],
                         start=True, stop=True)

    # ---- input DMAs (bf16 operands are cast host-side) ----
    # sync queue: xkv k-half 0; scalar queue: weights, xkv k-half 1, then
    # the residual (not needed until the first posts ~30us in)
    nc.sync.dma_start(xkv_h[:, 0, 0:XSPLIT], xkv_d[ts(0, 128), 0:XSPLIT])
    nc.sync.dma_start(xkv_h[:, 0, XSPLIT:N], xkv_d[ts(0, 128), XSPLIT:N])
    nc.scalar.dma_start(w12t[:, 0, :], w12_d[ts(0, 128), :])
    nc.scalar.dma_start(w12t[:, 1, :], w12_d[ts(1, 128), :])
    nc.scalar.dma_start(xkv_h[:, 1, 0:XSPLIT], xkv_d[ts(1, 128), 0:XSPLIT])
    nc.scalar.dma_start(w3t[:, 0, :], w3t_d[ts(0, 128), :])
    nc.scalar.dma_start(w3t[:, 1, :], w3t_d[ts(1, 128), :])
    nc.scalar.dma_start(xkv_h[:, 1, XSPLIT:N], xkv_d[ts(1, 128), XSPLIT:N])
    nc.scalar.dma_start(xqt[:], xqt_d[:])

    # ---- projections: f leads (queries = first 4 key chunks), then per
    # 512-col chunk one cycle of [S^T pair (lagging), g, V x4]; the f tail
    # is folded into the first loop iterations. S^T lags g by one chunk so
    # the PE never waits on the DVE g-copy; the V tiles keep the PE dense
    # while ACT drains the exps.
    Pt[0] = ptp.tile([128, MT, 512], BF16, tag="pt", name="pt")
    f_proj(0)
    f_proj(1)
    for ch in range(N // 512):
        if ch >= 1:
            st_group(0, 2 * (ch - 1))
            st_group(0, 2 * ch - 1)
        gp = op.tile([CK, 512], F32, tag="o", name="gp")
        for k in range(2):
            nc.tensor.matmul(gp[:], w12t[:, k, CK:2 * CK],
                             xkv_h[:, k, ts(ch, 512)],
                             start=(k == 0), stop=(k == 1))
        nc.vector.tensor_copy(g_sb[:, ts(ch, 512)], gp[:])
        if ch < 2:
            f_proj(ch + 2)
        for mt in range(4 * ch, 4 * ch + 4):
            vp = op.tile([128, 256], F32, tag="o", name="vp")
            for k in range(2):
                nc.tensor.matmul(vp[:], xkv_h[:, k, ts(mt, 128)], w3t[:, k, :],
                                 start=(k == 0), stop=(k == 1))
            nc.vector.tensor_copy(V[:, mt, 0:256], vp[:])
    st_group(0, 14)
    st_group(0, 15)

    # ---- attention chunks 1..NCHUNK, software-pipelined by one chunk ----
    for c in range(1, NCHUNK + 1):
        if c < NCHUNK:
            Pt[c % 2] = ptp.tile([128, MT, 512], BF16, tag="pt", name="pt")
        stage[(c - 1) % 2] = stgp.tile([128, 4, 256], F16, tag="stage", name="stage")
        o_cur = None
        for gidx in range(16):
            j, seg = gidx // 4, gidx % 4
            if seg == 0:
                o_cur = op.tile([128, 257], F32, tag="o", name="o")
            for mm in range(4):
                mt = seg * 8 + mm
                nc.tensor.matmul(o_cur[:], Pt[(c - 1) % 2][:, mt, ts(j, 128)],
                                 V[:, mt, 0:257],
                                 start=(mt == 0), stop=(mt == MT - 1),
                                 skip_group_check=True)
            # S^T groups in adjacent pairs: half as many PV<->S^T stream
            # transitions on the PE (each costs ~100ns of drained pipeline)
            if c < NCHUNK and gidx % 2 == 0:
                st_group(c, gidx)
                st_group(c, gidx + 1)
            for mm in range(4, 8):
                mt = seg * 8 + mm
                nc.tensor.matmul(o_cur[:], Pt[(c - 1) % 2][:, mt, ts(j, 128)],
                                 V[:, mt, 0:257],
                                 start=(mt == 0), stop=(mt == MT - 1),
                                 skip_group_check=True)
            if seg == 3:
                posts.append((c - 1, j, o_cur, stage[(c - 1) % 2]))
            # delay each n-tile's post-processing by one PE group so the DVE
            # normalize never stalls the PE stream; the final chunk has no
            # S^T stream left to protect, so flush immediately there
            while len(posts) > (1 if (gidx < 15 and c < NCHUNK) else 0):
                emit_post(posts.pop(0))
    while posts:
        emit_post(posts.pop(0))


_NC_CACHE = None


def _get_nc():
    global _NC_CACHE
    if _NC_CACHE is None:
        _NC_CACHE = build_nc()
    return _NC_CACHE


def make_in_maps(x, w1, w2, w3):
    x = np.ascontiguousarray(x, dtype=np.float32).reshape(B, C, N)
    w12 = np.ascontiguousarray(np.concatenate([w1.T, w2.T], axis=1)).astype(BF)
    w3t = np.ascontiguousarray(w3.T).astype(BF)
    in_maps = []
    xh = x.astype(BF)
    for core in range(8):
        b, half = core // 2, core % 2
        qsl = slice(half * NQ, (half + 1) * NQ)
        osl = slice((1 - half) * NQ, (2 - half) * NQ)
        # key axis permuted: own query columns first
        xkv = np.concatenate([xh[b][:, qsl], xh[b][:, osl]], axis=1)
        # residual for this core's queries: [NQ, C] -> [128, NJ, C]
        xqt = np.ascontiguousarray(
            x[b][:, qsl].T.astype(np.float16)
            .reshape(NJ, 128, C).transpose(1, 0, 2))
        in_maps.append({
            "xkv": np.ascontiguousarray(xkv),
            "xqt": xqt,
            "w12": w12,
            "w3t": w3t,
        })
    return in_maps


def assemble(results):
    out = np.empty((B, C, N), dtype=np.float32)
    for core in range(8):
        b, half = core // 2, core % 2
        o = np.asarray(results[core]["out"], dtype=np.float32)  # [128, NJ, C]
        out[b][:, half * NQ:(half + 1) * NQ] = (
            o.transpose(1, 0, 2).reshape(NQ, C).T)
    return out.reshape(B, C, H, W)


def kernel(x, w1, w2, w3):
    nc = _get_nc()
    res = run_bass_kernel_spmd(nc, make_in_maps(x, w1, w2, w3),
                               core_ids=list(range(8)))
    return assemble(res.results)
